# revision 8
# baseline (speedup 1.0000x reference)
"""Bahdanau-style attention scoring kernel for 8 TRN2 NeuronCores.

Reference computation (B=128, H=256, N=2048):
    hidden = concat([static, dynamic, broadcast(dec)], axis=1)   # [B, 3H, N]
    scores = tanh(einsum('hk,bkn->bhn', W[0], hidden))           # [B, H, N]
    logits = einsum('h,bhn->bn', v[0,0], scores)                 # [B, N]
    attns  = softmax(logits, axis=-1)[:, None, :]                # [B, 1, N]

The broadcast decoder term collapses to a per-batch bias vector
c[b] = W_dec @ dec[b] (precomputed on host, 0.003% of FLOPs), so per batch:
    scores_b = tanh(W_s @ static[b] + W_d @ dynamic[b] + c[b])

Sharding: data-parallel over batch, 16 batches per core; tiny W/v params
replicated (pre-cast to fp16 on host). No collectives needed.

Matmuls run in fp16 (1 cycle/row on the PE, f32 PSUM accumulate; verified
rel err ~2e-4). The 64MB/core of f32 activations is DMA'd raw and converted
f32->fp16 on-chip: static on GPSIMD, dynamic on the vector engine, both
otherwise idle.

The v-reduction uses a masked stationary matrix vm[p, b, m, j] =
v[m*128+p] * (j==b) so batch b's logits land on PSUM partition b,
accumulating all 16 batches into one [16, 512] PSUM tile per n-tile.
Softmax then runs batch-parallel on partitions 0..15.

This walrus build allows only ONE sync-wait per engine instruction, so the
kernel is structured so every instruction depends on at most one foreign
semaphore: per-k-tile simple DMAs into separate tiles, 1-element memset
"gate" ops that absorb buffer-reuse WAR waits, and cold-start priming ops
(dummy matmuls / a bias copy) that absorb first-use DMA waits.
"""

import sys

if "/opt/trn_rl_repo" not in sys.path:
    sys.path.insert(0, "/opt/trn_rl_repo")

import numpy as np

B, H, N = 128, 256, 2048
NCORES = 8
BPC = B // NCORES  # batches per core
P = 128            # SBUF partitions
KT = 4             # k-tiles over 2H=512 contraction
MT = 2             # m-tiles over H=256 output rows
NS = 512           # n-tile (one PSUM bank of f32)
NT = N // NS       # 4 n-tiles

_CACHE = {}


def _build():
    import concourse.bass as bass
    import concourse.bacc as bacc
    from concourse import mybir
    from concourse.tile import TileContext

    f32 = mybir.dt.float32
    f16 = mybir.dt.float16
    Tanh = mybir.ActivationFunctionType.Tanh
    Exp = mybir.ActivationFunctionType.Exp

    nc = bacc.Bacc()
    xs = nc.declare_dram_parameter("xs", [BPC, H, N], f32, isOutput=False)
    xd = nc.declare_dram_parameter("xd", [BPC, H, N], f32, isOutput=False)
    # wt[k, h] = W[h, k] for k in [0, 512): rows 0:256 static, 256:512 dynamic
    wt = nc.declare_dram_parameter("wt", [2 * H, H], f16, isOutput=False)
    # cb[h, b] = sum_k W[h, 512+k] * dec[b, k]  (host-precomputed bias)
    cb = nc.declare_dram_parameter("cb", [H, BPC], f32, isOutput=False)
    # vm[p, b, m, j] = v[m*128 + p] * (j == b)
    vm = nc.declare_dram_parameter("vm", [P, BPC, MT, BPC], f16, isOutput=False)
    out = nc.declare_dram_parameter("out", [BPC, N], f32, isOutput=True)

    with (
        TileContext(nc) as tc,
        tc.tile_pool(name="const", bufs=1) as cpool,
        tc.tile_pool(name="x", bufs=2) as xpool,
        tc.tile_pool(name="xh", bufs=2) as hpool,
        tc.tile_pool(name="sc", bufs=3) as spool,
        tc.tile_pool(name="ps", bufs=2, space="PSUM") as ppool,
        tc.tile_pool(name="pl", bufs=1, space="PSUM") as plpool,
    ):
        # --- replicated parameters, one simple DMA per separate tile ---
        wt_sb = []
        for kt in range(KT):
            w = cpool.tile([P, H], f16, name=f"wt{kt}", tag=f"wt{kt}")
            nc.sync.dma_start(out=w[:], in_=wt[kt * P:(kt + 1) * P, :])
            wt_sb.append(w)
        vm_sb = cpool.tile([P, BPC, MT, BPC], f16)
        nc.sync.dma_start(out=vm_sb[:], in_=vm[:])
        # bias laid out [128, m, b]
        c_sb = cpool.tile([P, MT, BPC], f32)
        nc.sync.dma_start(out=c_sb[:], in_=cb[:].rearrange("(m p) b -> p m b", p=P))

        # --- cold-start priming: absorb each param tile's DMA wait into a
        # throwaway op so steady-state instructions keep a single wait ---
        for kt in range(KT):
            pp = ppool.tile([P, 1], f32, name=f"prime{kt}", tag=f"ps{kt % MT}")
            nc.tensor.matmul(
                pp[:], lhsT=wt_sb[kt][:, :P], rhs=wt_sb[kt][:, 0:1],
                start=True, stop=True,
            )
        pv = ppool.tile([BPC, 1], f32, name="primev", tag="ps0")
        nc.tensor.matmul(
            pv[:], lhsT=vm_sb[:, 0, 0, :], rhs=vm_sb[:, 0, 0, 0:1],
            start=True, stop=True,
        )
        c_gate = cpool.tile([P, MT, BPC], f32)
        nc.scalar.copy(c_gate[:], c_sb[:])

        # logits accumulators: one [BPC, 512] PSUM tile per n-tile, written by
        # all 16 batches' masked v-matmuls (batch b lands on partition b)
        lp_tiles = [
            plpool.tile([BPC, NS], f32, tag=f"lp{nt}", name=f"lp{nt}")
            for nt in range(NT)
        ]

        # --- main loop: 16 batches ---
        for b in range(BPC):
            # per-k-tile raw f32 loads (simple 2D DMAs), then f32->fp16 on the
            # two otherwise-idle elementwise engines: static on GPSIMD,
            # dynamic on DVE. The 1-element memsets absorb the slot-reuse WAR
            # wait (on the PE semaphore) so each conversion carries only its
            # own DMA wait.
            xh = []
            for kt in range(2):
                xf = xpool.tile([P, N], f32, name=f"xsf{kt}", tag=f"xsf{kt}")
                nc.sync.dma_start(out=xf[:], in_=xs[b, kt * P:(kt + 1) * P, :])
                xc = hpool.tile([P, N], f16, name=f"xsh{kt}", tag=f"xsh{kt}")
                nc.gpsimd.memset(xc[0:1, 0:1], 0.0)
                nc.gpsimd.tensor_copy(xc[:], xf[:])
                xh.append(xc)
            for kt in range(2):
                xf = xpool.tile([P, N], f32, name=f"xdf{kt}", tag=f"xdf{kt}")
                nc.sync.dma_start(out=xf[:], in_=xd[b, kt * P:(kt + 1) * P, :])
                xc = hpool.tile([P, N], f16, name=f"xdh{kt}", tag=f"xdh{kt}")
                nc.vector.memset(xc[0:1, 0:1], 0.0)
                nc.vector.tensor_copy(xc[:], xf[:])
                xh.append(xc)
            for nt in range(NT):
                ns = slice(nt * NS, (nt + 1) * NS)
                sc_t = spool.tile([P, MT, NS], f16, tag="sc")
                for m in range(MT):
                    ps = ppool.tile([P, NS], f32, tag=f"ps{m}")
                    for kt in range(KT):
                        nc.tensor.matmul(
                            ps[:],
                            lhsT=wt_sb[kt][:, m * P:(m + 1) * P],
                            rhs=xh[kt][:, ns],
                            start=(kt == 0),
                            stop=(kt == KT - 1),
                        )
                    nc.scalar.activation(
                        sc_t[:, m, :], ps[:], Tanh,
                        bias=c_sb[:, m, b:b + 1],
                    )
                for m in range(MT):
                    nc.tensor.matmul(
                        lp_tiles[nt][:],
                        lhsT=vm_sb[:, b, m, :],
                        rhs=sc_t[:, m, :],
                        start=(b == 0 and m == 0),
                        stop=(b == BPC - 1 and m == MT - 1),
                    )

        # --- softmax over N per batch row (no max-subtraction: |logits| <~ 10) ---
        exp_sb = cpool.tile([BPC, N], f32)
        psums = cpool.tile([BPC, NT], f32)
        for nt in range(NT):
            nc.scalar.activation(
                exp_sb[:, nt * NS:(nt + 1) * NS], lp_tiles[nt][:], Exp,
                accum_out=psums[:, nt:nt + 1],
            )
        ssum = cpool.tile([BPC, 1], f32)
        nc.vector.reduce_sum(ssum[:], psums[:], axis=mybir.AxisListType.X)
        rec = cpool.tile([BPC, 1], f32)
        nc.vector.reciprocal(rec[:], ssum[:])
        att = cpool.tile([BPC, N], f32)
        nc.vector.tensor_scalar_mul(att[:], exp_sb[:], rec[:])
        nc.sync.dma_start(out=out[:], in_=att[:])

    nc.compile()
    return nc


def _make_in_maps(static_hidden, dynamic_hidden, decoder_hidden, v, W):
    W0 = np.asarray(W, dtype=np.float32)[0]          # [256, 768]
    wt_np = np.ascontiguousarray(W0[:, :2 * H].T.astype(np.float16))   # [512, 256]
    vhalf = np.asarray(v, dtype=np.float32)[0, 0].reshape(MT, P)       # [2, 128]
    # vm[p, b, m, j] = v[m*128+p] * (j == b)
    vm_np = np.ascontiguousarray(
        np.einsum("mp,bj->pbmj", vhalf, np.eye(BPC, dtype=np.float32))
        .astype(np.float16)
    )

    sh = np.asarray(static_hidden, dtype=np.float32)
    dh = np.asarray(dynamic_hidden, dtype=np.float32)
    dec = np.asarray(decoder_hidden, dtype=np.float32)
    # cb[h, b] = sum_k W_dec[h, k] dec[b, k], fp32 on host (tiny)
    cb_full = W0[:, 2 * H:] @ dec.T                  # [256, B]

    in_maps = []
    for i in range(NCORES):
        sl = slice(i * BPC, (i + 1) * BPC)
        in_maps.append({
            "xs": np.ascontiguousarray(sh[sl]),
            "xd": np.ascontiguousarray(dh[sl]),
            "wt": wt_np,
            "cb": np.ascontiguousarray(cb_full[:, sl]),
            "vm": vm_np,
        })
    return in_maps


def kernel(static_hidden, dynamic_hidden, decoder_hidden, v, W):
    from concourse.bass_utils import run_bass_kernel_spmd

    if "nc" not in _CACHE:
        _CACHE["nc"] = _build()
    nc = _CACHE["nc"]

    in_maps = _make_in_maps(static_hidden, dynamic_hidden, decoder_hidden, v, W)
    res = run_bass_kernel_spmd(nc, in_maps, core_ids=list(range(NCORES)))
    out = np.concatenate([r["out"] for r in res.results], axis=0)
    return out.reshape(B, 1, N).astype(np.float32)


# revision 10
# speedup vs baseline: 1.1492x; 1.1492x over previous
"""Bahdanau-style attention scoring kernel for 8 TRN2 NeuronCores.

Reference computation (B=128, H=256, N=2048):
    hidden = concat([static, dynamic, broadcast(dec)], axis=1)   # [B, 3H, N]
    scores = tanh(einsum('hk,bkn->bhn', W[0], hidden))           # [B, H, N]
    logits = einsum('h,bhn->bn', v[0,0], scores)                 # [B, N]
    attns  = softmax(logits, axis=-1)[:, None, :]                # [B, 1, N]

The broadcast decoder term collapses to a per-batch bias vector
c[b] = W_dec @ dec[b] (precomputed on host, 0.003% of FLOPs), so per batch:
    scores_b = tanh(W_s @ static[b] + W_d @ dynamic[b] + c[b])

Sharding: data-parallel over batch, 16 batches per core; tiny W/v params
replicated (pre-cast to bf16 on host). No collectives needed.

Matmuls run in bf16 (1 cycle/row on the PE, f32 PSUM accumulate; verified
rel err ~1.9e-3 vs the 2e-2 gate). The 64MB/core of f32 activations never
touches a compute engine on the way in: the DMA reads only the high 2 bytes
of each f32 (little-endian offset +2, stride 4) which IS bf16 truncation —
the cast is free.

Weight-stationary loop order (m -> kt -> nt) into a [128, 4x512] PSUM
supertile cuts LDWEIGHTS count 4x vs kt-inner order.

The v-reduction uses a masked stationary matrix vm[p, b, m, j] =
v[m*128+p] * (j==b) so batch b's logits land on PSUM partition b,
accumulating all 16 batches into one [16, 512] PSUM tile per n-tile.
Softmax then runs batch-parallel on partitions 0..15.
"""

import sys

if "/opt/trn_rl_repo" not in sys.path:
    sys.path.insert(0, "/opt/trn_rl_repo")

import numpy as np

B, H, N = 128, 256, 2048
NCORES = 8
BPC = B // NCORES  # batches per core
P = 128            # SBUF partitions
KT = 4             # k-tiles over 2H=512 contraction
MT = 2             # m-tiles over H=256 output rows
NS = 512           # n-tile (one PSUM bank of f32)
NT = N // NS       # 4 n-tiles

_CACHE = {}


def _build():
    import concourse.bacc as bacc
    from concourse import mybir
    from concourse.tile import TileContext

    f32 = mybir.dt.float32
    bf16 = mybir.dt.bfloat16
    Tanh = mybir.ActivationFunctionType.Tanh
    Exp = mybir.ActivationFunctionType.Exp

    nc = bacc.Bacc()
    xs = nc.declare_dram_parameter("xs", [BPC, H, N], f32, isOutput=False)
    xd = nc.declare_dram_parameter("xd", [BPC, H, N], f32, isOutput=False)
    # wt[k, h] = W[h, k] for k in [0, 512): rows 0:256 static, 256:512 dynamic
    wt = nc.declare_dram_parameter("wt", [2 * H, H], bf16, isOutput=False)
    # cb[h, b] = sum_k W[h, 512+k] * dec[b, k]  (host-precomputed bias)
    cb = nc.declare_dram_parameter("cb", [H, BPC], f32, isOutput=False)
    # vm[p, b, m, j] = v[m*128 + p] * (j == b)
    vm = nc.declare_dram_parameter("vm", [P, BPC, MT, BPC], bf16, isOutput=False)
    out = nc.declare_dram_parameter("out", [BPC, N], f32, isOutput=True)

    with (
        TileContext(nc) as tc,
        tc.tile_pool(name="const", bufs=1) as cpool,
        tc.tile_pool(name="xh", bufs=2) as hpool,
        tc.tile_pool(name="sc", bufs=2) as spool,
        tc.tile_pool(name="ps", bufs=1, space="PSUM") as ppool,
        tc.tile_pool(name="pl", bufs=1, space="PSUM") as plpool,
    ):
        # --- replicated parameters, one simple DMA per separate tile ---
        wt_sb = []
        for kt in range(KT):
            w = cpool.tile([P, H], bf16, name=f"wt{kt}", tag=f"wt{kt}")
            nc.sync.dma_start(out=w[:], in_=wt[kt * P:(kt + 1) * P, :])
            wt_sb.append(w)
        vm_sb = cpool.tile([P, BPC, MT, BPC], bf16)
        nc.sync.dma_start(out=vm_sb[:], in_=vm[:])
        # bias laid out [128, m, b]
        c_sb = cpool.tile([P, MT, BPC], f32)
        nc.sync.dma_start(out=c_sb[:], in_=cb[:].rearrange("(m p) b -> p m b", p=P))

        # logits accumulators: one [BPC, 512] PSUM tile per n-tile, written by
        # all 16 batches' masked v-matmuls (batch b lands on partition b)
        lp_tiles = [
            plpool.tile([BPC, NS], f32, tag=f"lp{nt}", name=f"lp{nt}")
            for nt in range(NT)
        ]

        # --- main loop: 16 batches ---
        for b in range(BPC):
            # raw f32 loads; the matmul rhs reads the high 2 bytes of each
            # f32 in SBUF (bf16 truncation) via a stride-2 bf16 AP, so no
            # compute engine ever touches the data on the way in.
            xh = []
            for kt in range(2):
                xf = hpool.tile([P, N], f32, name=f"xsf{kt}", tag=f"xsf{kt}")
                nc.sync.dma_start(out=xf[:], in_=xs[b, kt * P:(kt + 1) * P, :])
                xh.append(xf[:].bitcast(bf16)[:, 1::2])
            for kt in range(2):
                xf = hpool.tile([P, N], f32, name=f"xdf{kt}", tag=f"xdf{kt}")
                nc.sync.dma_start(out=xf[:], in_=xd[b, kt * P:(kt + 1) * P, :])
                xh.append(xf[:].bitcast(bf16)[:, 1::2])

            # weight-stationary matmuls into a 4-bank PSUM supertile
            sc_t = spool.tile([P, MT, N], bf16, tag="sc")
            for m in range(MT):
                pst = ppool.tile([P, NT, NS], f32, tag="pst")
                for kt in range(KT):
                    for nt in range(NT):
                        nc.tensor.matmul(
                            pst[:, nt, :],
                            lhsT=wt_sb[kt][:, m * P:(m + 1) * P],
                            rhs=xh[kt][:, nt * NS:(nt + 1) * NS],
                            start=(kt == 0),
                            stop=(kt == KT - 1),
                        )
                for nt in range(NT):
                    nc.scalar.activation(
                        sc_t[:, m, nt * NS:(nt + 1) * NS], pst[:, nt, :], Tanh,
                        bias=c_sb[:, m, b:b + 1],
                    )
            for nt in range(NT):
                for m in range(MT):
                    nc.tensor.matmul(
                        lp_tiles[nt][:],
                        lhsT=vm_sb[:, b, m, :],
                        rhs=sc_t[:, m, nt * NS:(nt + 1) * NS],
                        start=(b == 0 and m == 0),
                        stop=(b == BPC - 1 and m == MT - 1),
                    )

        # --- softmax over N per batch row (no max-subtraction: |logits| <~ 10) ---
        exp_sb = cpool.tile([BPC, N], f32)
        psums = cpool.tile([BPC, NT], f32)
        for nt in range(NT):
            nc.scalar.activation(
                exp_sb[:, nt * NS:(nt + 1) * NS], lp_tiles[nt][:], Exp,
                accum_out=psums[:, nt:nt + 1],
            )
        ssum = cpool.tile([BPC, 1], f32)
        nc.vector.reduce_sum(ssum[:], psums[:], axis=mybir.AxisListType.X)
        rec = cpool.tile([BPC, 1], f32)
        nc.vector.reciprocal(rec[:], ssum[:])
        att = cpool.tile([BPC, N], f32)
        nc.vector.tensor_scalar_mul(att[:], exp_sb[:], rec[:])
        nc.sync.dma_start(out=out[:], in_=att[:])

    nc.compile()
    return nc


def _make_in_maps(static_hidden, dynamic_hidden, decoder_hidden, v, W):
    import ml_dtypes

    bf16 = ml_dtypes.bfloat16
    W0 = np.asarray(W, dtype=np.float32)[0]          # [256, 768]
    wt_np = np.ascontiguousarray(W0[:, :2 * H].T.astype(bf16))   # [512, 256]
    vhalf = np.asarray(v, dtype=np.float32)[0, 0].reshape(MT, P)  # [2, 128]
    # vm[p, b, m, j] = v[m*128+p] * (j == b)
    vm_np = np.ascontiguousarray(
        np.einsum("mp,bj->pbmj", vhalf, np.eye(BPC, dtype=np.float32))
        .astype(bf16)
    )

    sh = np.asarray(static_hidden, dtype=np.float32)
    dh = np.asarray(dynamic_hidden, dtype=np.float32)
    dec = np.asarray(decoder_hidden, dtype=np.float32)
    # cb[h, b] = sum_k W_dec[h, k] dec[b, k], fp32 on host (tiny)
    cb_full = W0[:, 2 * H:] @ dec.T                  # [256, B]

    in_maps = []
    for i in range(NCORES):
        sl = slice(i * BPC, (i + 1) * BPC)
        in_maps.append({
            "xs": np.ascontiguousarray(sh[sl]),
            "xd": np.ascontiguousarray(dh[sl]),
            "wt": wt_np,
            "cb": np.ascontiguousarray(cb_full[:, sl]),
            "vm": vm_np,
        })
    return in_maps


def kernel(static_hidden, dynamic_hidden, decoder_hidden, v, W):
    from concourse.bass_utils import run_bass_kernel_spmd

    if "nc" not in _CACHE:
        _CACHE["nc"] = _build()
    nc = _CACHE["nc"]

    in_maps = _make_in_maps(static_hidden, dynamic_hidden, decoder_hidden, v, W)
    res = run_bass_kernel_spmd(nc, in_maps, core_ids=list(range(NCORES)))
    out = np.concatenate([r["out"] for r in res.results], axis=0)
    return out.reshape(B, 1, N).astype(np.float32)


# revision 11
# speedup vs baseline: 1.1675x; 1.0159x over previous
"""Bahdanau-style attention scoring kernel for 8 TRN2 NeuronCores.

Reference computation (B=128, H=256, N=2048):
    hidden = concat([static, dynamic, broadcast(dec)], axis=1)   # [B, 3H, N]
    scores = tanh(einsum('hk,bkn->bhn', W[0], hidden))           # [B, H, N]
    logits = einsum('h,bhn->bn', v[0,0], scores)                 # [B, N]
    attns  = softmax(logits, axis=-1)[:, None, :]                # [B, 1, N]

The broadcast decoder term collapses to a per-batch bias vector
c[b] = W_dec @ dec[b] (precomputed on host, 0.003% of FLOPs), so per batch:
    scores_b = tanh(W_s @ static[b] + W_d @ dynamic[b] + c[b])

Sharding: data-parallel over batch, 16 batches per core; tiny W/v params
replicated (pre-cast to bf16 on host). No collectives needed.

Matmuls run in bf16 (1 cycle/row on the PE, f32 PSUM accumulate; verified
rel err ~1.9e-3 vs the 2e-2 gate). The 64MB/core of f32 activations never
touches a compute engine on the way in: the DMA reads only the high 2 bytes
of each f32 (little-endian offset +2, stride 4) which IS bf16 truncation —
the cast is free.

Weight-stationary loop order (m -> kt -> nt) into a [128, 4x512] PSUM
supertile cuts LDWEIGHTS count 4x vs kt-inner order.

The v-reduction uses a masked stationary matrix vm[p, b, m, j] =
v[m*128+p] * (j==b) so batch b's logits land on PSUM partition b,
accumulating all 16 batches into one [16, 512] PSUM tile per n-tile.
Softmax then runs batch-parallel on partitions 0..15.
"""

import sys

if "/opt/trn_rl_repo" not in sys.path:
    sys.path.insert(0, "/opt/trn_rl_repo")

import numpy as np

B, H, N = 128, 256, 2048
NCORES = 8
BPC = B // NCORES  # batches per core
P = 128            # SBUF partitions
KT = 4             # k-tiles over 2H=512 contraction
MT = 2             # m-tiles over H=256 output rows
NS = 512           # n-tile (one PSUM bank of f32)
NT = N // NS       # 4 n-tiles

_CACHE = {}


def _build():
    import concourse.bacc as bacc
    from concourse import mybir
    from concourse.tile import TileContext

    f32 = mybir.dt.float32
    bf16 = mybir.dt.bfloat16
    Tanh = mybir.ActivationFunctionType.Tanh
    Exp = mybir.ActivationFunctionType.Exp

    nc = bacc.Bacc()
    xs = nc.declare_dram_parameter("xs", [BPC, H, N], f32, isOutput=False)
    xd = nc.declare_dram_parameter("xd", [BPC, H, N], f32, isOutput=False)
    # wt[k, h] = W[h, k] for k in [0, 512): rows 0:256 static, 256:512 dynamic
    wt = nc.declare_dram_parameter("wt", [2 * H, H], bf16, isOutput=False)
    # cb[h, b] = sum_k W[h, 512+k] * dec[b, k]  (host-precomputed bias)
    cb = nc.declare_dram_parameter("cb", [H, BPC], f32, isOutput=False)
    # vm[p, b, m, j] = v[m*128 + p] * (j == b)
    vm = nc.declare_dram_parameter("vm", [P, BPC, MT, BPC], bf16, isOutput=False)
    out = nc.declare_dram_parameter("out", [BPC, N], f32, isOutput=True)

    with (
        TileContext(nc) as tc,
        tc.tile_pool(name="const", bufs=1) as cpool,
        tc.tile_pool(name="xh", bufs=3) as hpool,
        tc.tile_pool(name="sc", bufs=2) as spool,
        tc.tile_pool(name="ps", bufs=1, space="PSUM") as ppool,
        tc.tile_pool(name="pl", bufs=1, space="PSUM") as plpool,
    ):
        # --- replicated parameters, one simple DMA per separate tile ---
        wt_sb = []
        for kt in range(KT):
            w = cpool.tile([P, H], bf16, name=f"wt{kt}", tag=f"wt{kt}")
            nc.sync.dma_start(out=w[:], in_=wt[kt * P:(kt + 1) * P, :])
            wt_sb.append(w)
        vm_sb = cpool.tile([P, BPC, MT, BPC], bf16)
        nc.sync.dma_start(out=vm_sb[:], in_=vm[:])
        # bias laid out [128, m, b]
        c_sb = cpool.tile([P, MT, BPC], f32)
        nc.sync.dma_start(out=c_sb[:], in_=cb[:].rearrange("(m p) b -> p m b", p=P))

        # logits accumulators: one [BPC, 512] PSUM tile per n-tile, written by
        # all 16 batches' masked v-matmuls (batch b lands on partition b)
        lp_tiles = [
            plpool.tile([BPC, NS], f32, tag=f"lp{nt}", name=f"lp{nt}")
            for nt in range(NT)
        ]

        # --- main loop: 16 batches; v-matmuls are software-pipelined one
        # batch behind the main matmuls so the PE never waits on the
        # scalar engine's tanh ---
        sc_hist = {}

        def emit_vmms(vb):
            sc_prev = sc_hist.pop(vb)
            for nt in range(NT):
                for m in range(MT):
                    nc.tensor.matmul(
                        lp_tiles[nt][:],
                        lhsT=vm_sb[:, vb, m, :],
                        rhs=sc_prev[:, m, nt * NS:(nt + 1) * NS],
                        start=(vb == 0 and m == 0),
                        stop=(vb == BPC - 1 and m == MT - 1),
                    )

        for b in range(BPC):
            # raw f32 loads; the matmul rhs reads the high 2 bytes of each
            # f32 in SBUF (bf16 truncation) via a stride-2 bf16 AP, so no
            # compute engine ever touches the data on the way in.
            xh = []
            for kt in range(2):
                xf = hpool.tile([P, N], f32, name=f"xsf{kt}", tag=f"xsf{kt}")
                nc.sync.dma_start(out=xf[:], in_=xs[b, kt * P:(kt + 1) * P, :])
                xh.append(xf[:].bitcast(bf16)[:, 1::2])
            for kt in range(2):
                xf = hpool.tile([P, N], f32, name=f"xdf{kt}", tag=f"xdf{kt}")
                nc.sync.dma_start(out=xf[:], in_=xd[b, kt * P:(kt + 1) * P, :])
                xh.append(xf[:].bitcast(bf16)[:, 1::2])

            # weight-stationary matmuls into a 4-bank PSUM supertile
            sc_t = spool.tile([P, MT, N], bf16, tag="sc")
            for m in range(MT):
                pst = ppool.tile([P, NT, NS], f32, tag="pst")
                for kt in range(KT):
                    for nt in range(NT):
                        nc.tensor.matmul(
                            pst[:, nt, :],
                            lhsT=wt_sb[kt][:, m * P:(m + 1) * P],
                            rhs=xh[kt][:, nt * NS:(nt + 1) * NS],
                            start=(kt == 0),
                            stop=(kt == KT - 1),
                        )
                for nt in range(NT):
                    nc.scalar.activation(
                        sc_t[:, m, nt * NS:(nt + 1) * NS], pst[:, nt, :], Tanh,
                        bias=c_sb[:, m, b:b + 1],
                    )
            sc_hist[b] = sc_t
            if b > 0:
                emit_vmms(b - 1)
        emit_vmms(BPC - 1)

        # --- softmax over N per batch row (no max-subtraction: |logits| <~ 10) ---
        exp_sb = cpool.tile([BPC, N], f32)
        psums = cpool.tile([BPC, NT], f32)
        for nt in range(NT):
            nc.scalar.activation(
                exp_sb[:, nt * NS:(nt + 1) * NS], lp_tiles[nt][:], Exp,
                accum_out=psums[:, nt:nt + 1],
            )
        ssum = cpool.tile([BPC, 1], f32)
        nc.vector.reduce_sum(ssum[:], psums[:], axis=mybir.AxisListType.X)
        rec = cpool.tile([BPC, 1], f32)
        nc.vector.reciprocal(rec[:], ssum[:])
        att = cpool.tile([BPC, N], f32)
        nc.vector.tensor_scalar_mul(att[:], exp_sb[:], rec[:])
        nc.sync.dma_start(out=out[:], in_=att[:])

    nc.compile()
    return nc


def _make_in_maps(static_hidden, dynamic_hidden, decoder_hidden, v, W):
    import ml_dtypes

    bf16 = ml_dtypes.bfloat16
    W0 = np.asarray(W, dtype=np.float32)[0]          # [256, 768]
    wt_np = np.ascontiguousarray(W0[:, :2 * H].T.astype(bf16))   # [512, 256]
    vhalf = np.asarray(v, dtype=np.float32)[0, 0].reshape(MT, P)  # [2, 128]
    # vm[p, b, m, j] = v[m*128+p] * (j == b)
    vm_np = np.ascontiguousarray(
        np.einsum("mp,bj->pbmj", vhalf, np.eye(BPC, dtype=np.float32))
        .astype(bf16)
    )

    sh = np.asarray(static_hidden, dtype=np.float32)
    dh = np.asarray(dynamic_hidden, dtype=np.float32)
    dec = np.asarray(decoder_hidden, dtype=np.float32)
    # cb[h, b] = sum_k W_dec[h, k] dec[b, k], fp32 on host (tiny)
    cb_full = W0[:, 2 * H:] @ dec.T                  # [256, B]

    in_maps = []
    for i in range(NCORES):
        sl = slice(i * BPC, (i + 1) * BPC)
        in_maps.append({
            "xs": np.ascontiguousarray(sh[sl]),
            "xd": np.ascontiguousarray(dh[sl]),
            "wt": wt_np,
            "cb": np.ascontiguousarray(cb_full[:, sl]),
            "vm": vm_np,
        })
    return in_maps


def kernel(static_hidden, dynamic_hidden, decoder_hidden, v, W):
    from concourse.bass_utils import run_bass_kernel_spmd

    if "nc" not in _CACHE:
        _CACHE["nc"] = _build()
    nc = _CACHE["nc"]

    in_maps = _make_in_maps(static_hidden, dynamic_hidden, decoder_hidden, v, W)
    res = run_bass_kernel_spmd(nc, in_maps, core_ids=list(range(NCORES)))
    out = np.concatenate([r["out"] for r in res.results], axis=0)
    return out.reshape(B, 1, N).astype(np.float32)
